# revision 8
# baseline (speedup 1.0000x reference)
"""GNN (GraphConv -> SAGEConv -> ChebConv) Bass kernel for 8 trn2 NeuronCores.

Strategy (graph/data parallel, dst-sharded):
- Node range sharded 8 ways; core c owns dst nodes [c*12500, (c+1)*12500), padded
  to 12544 local slots.
- Each layer's activations are republished to all cores via AllGather into a
  full [100352, 128] gather table in HBM (row = global padded node id).
- SpMM agg(x)[dst] = sum_{edges} w_e * x[src]: edges sorted by dst, bucketed by
  (superwindow of 1024 dst nodes) x (table quarter of 25088 rows, so int16
  indices address it). Per bucket one dma_gather (custom SWDGE ucode, 4 queues)
  pulls 128-edge chunks of 512B rows; DVE builds a weighted one-hot
  S[e, j] = (dstloc[e]==j) * w[e]; PE accumulates G^T @ S into a feat-major
  PSUM superwindow at a per-chunk dynamic column offset (register-indexed AP).
  All graph normalizations are folded into the per-edge weights w.
- Dense layers / bias+relu / transposes to node-major / table writes are
  streamed per superwindow, feat-major, on PE/ACT/DVE under the gather shadow.
"""
import sys
sys.path.insert(0, "/opt/trn_rl_repo")

import numpy as np

P = 128

CFG = dict(
    N=100000, E=1600000, INF=128, HID=128, OUTF=64,
    NCORES=8, SWN=1024, SPAN=64,
)


def _derive(cfg):
    d = dict(cfg)
    d["V"] = d["N"] // d["NCORES"]                      # real nodes per core
    d["VP"] = ((d["V"] + P - 1) // P) * P               # padded
    d["ROWS"] = d["VP"] * d["NCORES"]                   # table rows
    npair = d.get("FORCE_NPAIR") or (d["ROWS"] + 32767) // 32768
    d["NPAIR"] = npair
    d["PAIR"] = ((d["ROWS"] + npair - 1) // npair + P - 1) // P * P
    d["ROWSA"] = d["PAIR"] * npair
    d["NSW"] = (d["VP"] + d["SWN"] - 1) // d["SWN"]
    return d


def _chunkify(dl_rel, sw_width, span):
    """Greedy chunks over dst-sorted local-in-SW positions dl_rel (ascending).
    Each chunk: <=128 edges, dst span < span, [d0, d0+span) within one 512-col
    psum bank (and within sw_width for the last partial superwindow).
    Returns list of (start, end, d0)."""
    out = []
    i, n = 0, len(dl_rel)
    while i < n:
        d0 = int(dl_rel[i])
        bank = d0 // 512
        bank_end = min((bank + 1) * 512, max(sw_width, d0 + 1))
        d0 = min(d0, max(bank * 512, bank_end - span))
        hi = min(d0 + span, bank_end)
        j = i
        while j < n and j - i < P and dl_rel[j] < hi:
            j += 1
        out.append((i, j, d0))
        i = j
    return out


def preprocess(src, dst, cfg):
    """Build per-core static schedule + per-core data arrays.

    Returns (NCH, percore) where NCH[sw][p] is the static chunk count and
    percore[c] is a dict of numpy arrays for core c.
    """
    c_ = _derive(cfg)
    NCORES, V, VP, SWN, SPAN, NSW, NPAIR, PAIR = (
        c_["NCORES"], c_["V"], c_["VP"], c_["SWN"], c_["SPAN"], c_["NSW"],
        c_["NPAIR"], c_["PAIR"])
    N = c_["N"]

    src = np.asarray(src).astype(np.int64)
    dst = np.asarray(dst).astype(np.int64)
    deg_in = np.bincount(dst, minlength=N).astype(np.float32)
    deg_out = np.bincount(src, minlength=N).astype(np.float32)
    norm_in = np.maximum(deg_in, 1.0) ** -0.5
    norm_out = np.maximum(deg_out, 1.0) ** -0.5
    inv_deg = 1.0 / np.maximum(deg_in, 1.0)
    nrm = norm_in

    core_of = dst // V
    srow = (src // V) * VP + (src % V)          # padded-table row of src
    pair_of = srow // PAIR
    sloc = srow % PAIR

    # per-core, per-(sw, pair): sorted edge lists
    buckets = [[[None] * NPAIR for _ in range(NSW)] for _ in range(NCORES)]
    chunked = [[[None] * NPAIR for _ in range(NSW)] for _ in range(NCORES)]
    for c in range(NCORES):
        sel = np.nonzero(core_of == c)[0]
        dl = dst[sel] - c * V
        sw_of = dl // SWN
        for swi in range(NSW):
            m1 = sw_of == swi
            sw_width = min(SWN, VP - swi * SWN)
            for p in range(NPAIR):
                m = m1 & (pair_of[sel] == p)
                eids = sel[m]
                order = np.argsort(dl[m], kind="stable")
                eids = eids[order]
                dl_rel = (dst[eids] - c * V - swi * SWN).astype(np.int64)
                buckets[c][swi][p] = (eids, dl_rel)
                chunked[c][swi][p] = _chunkify(dl_rel, sw_width, SPAN)

    NCH = [[0] * NPAIR for _ in range(NSW)]
    for swi in range(NSW):
        for p in range(NPAIR):
            NCH[swi][p] = max(len(chunked[c][swi][p]) for c in range(NCORES))
    CT = sum(sum(r) for r in NCH)

    # weights per edge for the 4 SpMMs
    w_all = np.stack([
        norm_out[src] * norm_in[dst],            # L1 GraphConv both-norm
        inv_deg[dst],                            # L2 SAGE mean
        -(nrm[src] * nrm[dst]),                  # L3 X1 = -Ahat(X0)
        -2.0 * (nrm[src] * nrm[dst]),            # L3 X2 += -2 Ahat(X1)
    ]).astype(np.float32)

    percore = []
    for c in range(NCORES):
        idx_blocks = []
        dstloc = np.full((CT, P), -1.0, np.float32)
        d0a = np.zeros((CT,), np.int32)
        wa = np.zeros((4, CT, P), np.float32)
        cgl = 0
        for swi in range(NSW):
            for p in range(NPAIR):
                nch = NCH[swi][p]
                if nch == 0:
                    continue
                eids, dl_rel = buckets[c][swi][p]
                chunks = chunked[c][swi][p]
                idx = np.zeros((nch * P,), np.int16)
                for j in range(nch):
                    if j < len(chunks):
                        i0, i1, d0 = chunks[j]
                        k = i1 - i0
                        e = eids[i0:i1]
                        idx[j * P : j * P + k] = sloc[e].astype(np.int16)
                        dstloc[cgl + j, :k] = (dl_rel[i0:i1] - d0).astype(np.float32)
                        d0a[cgl + j] = d0
                        wa[:, cgl + j, :k] = w_all[:, e]
                # wrap col-major into 16 channels, replicate to 128 partitions
                blk = idx.reshape(-1, 16).T          # [16, nch*8]
                idx_blocks.append(np.tile(blk, (8, 1)))
                cgl += nch
        assert cgl == CT
        percore.append(dict(
            idx16=np.concatenate(idx_blocks, axis=1).astype(np.int16),
            dstloc=np.ascontiguousarray(dstloc.T),        # [128, CT]
            d0=d0a.reshape(1, CT),
            w0=np.ascontiguousarray(wa[0].T), w1=np.ascontiguousarray(wa[1].T),
            w2=np.ascontiguousarray(wa[2].T), w3=np.ascontiguousarray(wa[3].T),
        ))
    return c_, NCH, CT, percore


def _patch_lane_assignment():
    """Make Tile's DMASW completion-lane rotation consistent with our SWDGE
    queue_num rotation: a semaphore lane must only ever be updated from one
    SWDGE queue, so route queue q to lanes {q, q+4}."""
    import concourse.tile_sem_assignment as tsa
    import concourse.bass_isa as bass_isa
    import concourse.mybir as mybir
    if getattr(tsa.TileClockTick, "_gnn_patched", False):
        return
    orig = tsa.TileClockTick._assign_tick

    def patched(self, inst):
        if (isinstance(inst, tsa.DMAInst)
                and not isinstance(inst, bass_isa.UserSyncedRemoteDMADescs)
                and inst.engine == mybir.EngineType.Pool):
            q = int(getattr(inst, "queue_num", 0) or 0)
            cnt = getattr(self, "_gnn_qcnt", None)
            if cnt is None:
                cnt = self._gnn_qcnt = {}
            k = cnt.get(q, 0)
            cnt[q] = k + 1
            self.next_sw_dma_idx = (q + 4 * (k % 2)) % self.swdge_sem_count
        return orig(self, inst)

    tsa.TileClockTick._assign_tick = patched
    tsa.TileClockTick._gnn_patched = True


def build_kernel(c_, NCH, CT):
    import concourse.bass as bass
    import concourse.bacc as bacc
    import concourse.mybir as mybir
    import concourse.tile as tile
    from concourse import library_config
    from concourse.masks import make_identity
    from concourse.tile_rust import add_dep_helper

    _patch_lane_assignment()

    NCORES, VP, ROWS, SWN, SPAN, NSW, NPAIR, PAIR = (
        c_["NCORES"], c_["VP"], c_["ROWS"], c_["SWN"], c_["SPAN"], c_["NSW"],
        c_["NPAIR"], c_["PAIR"])
    HID, OUTF = c_["HID"], c_["OUTF"]
    f32 = mybir.dt.float32
    PE = mybir.EngineType.PE
    eq, mul, sub = (mybir.AluOpType.is_equal, mybir.AluOpType.mult,
                    mybir.AluOpType.subtract)
    Relu = mybir.ActivationFunctionType.Relu
    Ident = mybir.ActivationFunctionType.Identity

    nc = bacc.Bacc("TRN2", target_bir_lowering=False, debug=False,
                   num_devices=NCORES, num_swdge_queues=4)

    feat_in = nc.dram_tensor("feat_shard", [VP, 128], f32, kind="ExternalInput")
    idx_in = nc.dram_tensor("idx16", [P, CT * 8], mybir.dt.int16, kind="ExternalInput")
    dstloc_in = nc.dram_tensor("dstloc", [P, CT], f32, kind="ExternalInput")
    d0_in = nc.dram_tensor("d0", [1, CT], mybir.dt.int32, kind="ExternalInput")
    w_in = [nc.dram_tensor(f"w{i}", [P, CT], f32, kind="ExternalInput") for i in range(4)]
    W1_in = nc.dram_tensor("W1", [128, HID], f32, kind="ExternalInput")
    Ws_in = nc.dram_tensor("W_self", [HID, HID], f32, kind="ExternalInput")
    Wn_in = nc.dram_tensor("W_neigh", [HID, HID], f32, kind="ExternalInput")
    Wc_in = nc.dram_tensor("W_cheb3", [128, 3, OUTF], f32, kind="ExternalInput")
    b1_in = nc.dram_tensor("b1", [HID, 1], f32, kind="ExternalInput")
    b2_in = nc.dram_tensor("b2", [HID, 1], f32, kind="ExternalInput")
    b3_in = nc.dram_tensor("b3", [OUTF, 1], f32, kind="ExternalInput")
    iota_in = nc.dram_tensor("iota", [P, SPAN], f32, kind="ExternalInput")
    out_dram = nc.dram_tensor("out", [VP, OUTF], f32, kind="ExternalOutput")

    rg = [list(range(NCORES))]

    with tile.TileContext(nc) as tc:
        with (
            tc.tile_pool(name="dram", bufs=1, space="DRAM") as dpool,
            tc.tile_pool(name="const", bufs=1) as cpool,
            tc.tile_pool(name="big", bufs=1) as bigpool,
            tc.tile_pool(name="gp", bufs=2) as gpool,
            tc.tile_pool(name="idxp", bufs=3) as idxpool,
            tc.tile_pool(name="sp", bufs=8) as spool,
            tc.tile_pool(name="slice", bufs=2) as slpool,
            tc.tile_pool(name="nmp", bufs=2) as nmpool,
            tc.tile_pool(name="wp", bufs=1) as wpool,
            tc.tile_pool(name="pssw", bufs=2, space="PSUM") as ps_sw,
            tc.tile_pool(name="psd", bufs=2, space="PSUM") as ps_d,
            tc.tile_pool(name="pst", bufs=2, space="PSUM") as ps_t,
        ):
            lib = nc.gpsimd.load_library(library_config.mlp)

            ROWSA = c_["ROWSA"]
            tables = [dpool.tile([ROWSA, 128], f32, addr_space="Shared",
                                 name=f"T{i}") for i in range(4)]
            bounces = [dpool.tile([VP, 128], f32, name=f"bounce{i}")
                       for i in range(4)]

            iota = cpool.tile([P, SPAN], f32)
            nc.sync.dma_start(iota[:], iota_in[:])
            ident = cpool.tile([P, P], f32)
            make_identity(nc, ident[:])
            z512 = cpool.tile([P, 512], f32)
            nc.vector.memset(z512[:], 0.0)
            W1sb = cpool.tile([128, HID], f32); nc.sync.dma_start(W1sb[:], W1_in[:])
            Wssb = cpool.tile([HID, HID], f32); nc.sync.dma_start(Wssb[:], Ws_in[:])
            Wnsb = cpool.tile([HID, HID], f32); nc.sync.dma_start(Wnsb[:], Wn_in[:])
            Wcsb = cpool.tile([128, 3, OUTF], f32); nc.sync.dma_start(Wcsb[:], Wc_in[:])
            b1sb = cpool.tile([HID, 1], f32); nc.sync.dma_start(b1sb[:], b1_in[:])
            b2sb = cpool.tile([HID, 1], f32); nc.sync.dma_start(b2sb[:], b2_in[:])
            b3sb = cpool.tile([OUTF, 1], f32); nc.sync.dma_start(b3sb[:], b3_in[:])
            dstloc = cpool.tile([P, CT], f32); nc.sync.dma_start(dstloc[:], dstloc_in[:])
            d0t = cpool.tile([1, CT], mybir.dt.int32); nc.sync.dma_start(d0t[:], d0_in[:])

            h1T = bigpool.tile([P, VP], f32)     # L1 out; reused as X1T in L3
            h2T = bigpool.tile([P, VP], f32)     # L2 out = X0
            X1T = h1T

            # T0 = AllGather of raw feature shards
            nc.gpsimd.dma_start(bounces[0][:], feat_in[:])
            nc.gpsimd.collective_compute(
                "AllGather", mybir.AluOpType.bypass, replica_groups=rg,
                ins=[bounces[0][:]], outs=[tables[0][:ROWS, :]])

            qrot = [0]

            def spmm_sw(l, swi, wbuf, coff):
                """Run the SpMM for superwindow swi of layer l; returns psum."""
                ps = ps_sw.tile([P, SWN], f32, tag="sw")
                nbank = (SWN + 511) // 512
                for b in range(nbank):
                    bw = min(512, SWN - b * 512)
                    nc.tensor.matmul(out=ps[:, b * 512 : b * 512 + bw],
                                     lhsT=z512[:, :128], rhs=z512[:, :bw],
                                     start=True, stop=False)
                c = coff
                for p in range(NPAIR):
                    nch = NCH[swi][p]
                    if nch == 0:
                        continue
                    ioff = c * 8
                    idxs = idxpool.tile([P, nch * 8], mybir.dt.int16, tag="idx")
                    nc.sync.dma_start(idxs[:], idx_in[:, ioff : ioff + nch * 8])
                    G = gpool.tile([P, nch, 128], f32, tag="g")
                    gi = nc.gpsimd.dma_gather(
                        G[:], tables[l][p * PAIR : (p + 1) * PAIR, :], idxs[:],
                        nch * P, nch * P, 128, single_packet=False,
                        queue_num=qrot[0] % 4)
                    qrot[0] += 1
                    add_dep_helper(gi.ins, lib.ins, sync=False,
                                   reason="lib before gather")
                    vals = None
                    for j in range(nch):
                        if j % 8 == 0:
                            k = min(8, nch - j)
                            _, vals = nc.values_load_multi_w_load_instructions(
                                d0t[0:1, c + j : c + j + k], engines=(PE,),
                                min_val=0, max_val=SWN - SPAN,
                                skip_runtime_bounds_check=True)
                        S = spool.tile([P, SPAN], f32, tag="s")
                        nc.vector.scalar_tensor_tensor(
                            S[:], iota[:], dstloc[:, c + j : c + j + 1],
                            wbuf[:, c + j : c + j + 1].to_broadcast([P, SPAN]),
                            eq, mul)
                        nc.tensor.matmul(
                            out=ps[:, bass.ds(vals[j % 8], SPAN)],
                            lhsT=G[:, j, :], rhs=S[:],
                            start=False, stop=False, skip_group_check=True)
                    c += nch
                for b in range(nbank):
                    bw = min(512, SWN - b * 512)
                    nc.tensor.matmul(out=ps[:, b * 512 : b * 512 + bw],
                                     lhsT=z512[:, :128], rhs=z512[:, :bw],
                                     start=False, stop=True)
                return ps, c

            def table_write(hT, s0, wd, bounce):
                n128 = wd // P
                nm = nmpool.tile([P, SWN // P, P], f32, tag="nm")
                for w8 in range(n128):
                    pt = ps_t.tile([P, P], f32, tag="t")
                    nc.tensor.transpose(out=pt[:], in_=hT[:, s0 + w8 * P : s0 + (w8 + 1) * P],
                                        identity=ident[:])
                    nc.vector.tensor_copy(nm[:, w8, :], pt[:])
                nc.sync.dma_start(
                    bounce[s0 : s0 + wd, :].rearrange("(w p) f -> p w f", p=P),
                    nm[:, :n128, :])

            coff = 0
            layer_w = []
            for l in range(4):
                wb = wpool.tile([P, CT], f32, tag="w")
                nc.sync.dma_start(wb[:], w_in[l][:])
                layer_w.append(wb)

            # ---------------- Layer 1 ----------------
            coff = 0
            for swi in range(NSW):
                s0 = swi * SWN
                wd = min(SWN, VP - s0)
                ps, coff = spmm_sw(0, swi, layer_w[0], coff)
                agg = slpool.tile([P, SWN], f32, tag="swsl")
                nc.vector.tensor_copy(agg[:, :wd], ps[:, :wd])
                for t in range((wd + 511) // 512):
                    w512 = min(512, wd - t * 512)
                    pd = ps_d.tile([P, 512], f32, tag="d")
                    nc.tensor.matmul(out=pd[:, :w512], lhsT=W1sb[:],
                                     rhs=agg[:, t * 512 : t * 512 + w512],
                                     start=True, stop=True)
                    nc.scalar.activation(h1T[:, s0 + t * 512 : s0 + t * 512 + w512],
                                         pd[:, :w512], Relu, bias=b1sb[:])
                table_write(h1T, s0, wd, bounces[1])
            nc.gpsimd.collective_compute(
                "AllGather", mybir.AluOpType.bypass, replica_groups=rg,
                ins=[bounces[1][:]], outs=[tables[1][:ROWS, :]])

            # ---------------- Layer 2 ----------------
            coff = 0
            for swi in range(NSW):
                s0 = swi * SWN
                wd = min(SWN, VP - s0)
                ps, coff = spmm_sw(1, swi, layer_w[1], coff)
                agg = slpool.tile([P, SWN], f32, tag="swsl")
                nc.vector.tensor_copy(agg[:, :wd], ps[:, :wd])
                for t in range((wd + 511) // 512):
                    w512 = min(512, wd - t * 512)
                    pd = ps_d.tile([P, 512], f32, tag="d")
                    nc.tensor.matmul(out=pd[:, :w512], lhsT=Wssb[:],
                                     rhs=h1T[:, s0 + t * 512 : s0 + t * 512 + w512],
                                     start=True, stop=False)
                    nc.tensor.matmul(out=pd[:, :w512], lhsT=Wnsb[:],
                                     rhs=agg[:, t * 512 : t * 512 + w512],
                                     start=False, stop=True)
                    nc.scalar.activation(h2T[:, s0 + t * 512 : s0 + t * 512 + w512],
                                         pd[:, :w512], Relu, bias=b2sb[:])
                table_write(h2T, s0, wd, bounces[2])
            nc.gpsimd.collective_compute(
                "AllGather", mybir.AluOpType.bypass, replica_groups=rg,
                ins=[bounces[2][:]], outs=[tables[2][:ROWS, :]])

            # ---------------- Layer 3a: X1 = -Ahat(X0) ----------------
            coff = 0
            for swi in range(NSW):
                s0 = swi * SWN
                wd = min(SWN, VP - s0)
                ps, coff = spmm_sw(2, swi, layer_w[2], coff)
                nc.vector.tensor_copy(X1T[:, s0 : s0 + wd], ps[:, :wd])
                table_write(X1T, s0, wd, bounces[3])
            nc.gpsimd.collective_compute(
                "AllGather", mybir.AluOpType.bypass, replica_groups=rg,
                ins=[bounces[3][:]], outs=[tables[3][:ROWS, :]])

            # ------- Layer 3b: X2 = -2 Ahat(X1) - X0; out = Xt @ Wc + b3 -----
            coff = 0
            for swi in range(NSW):
                s0 = swi * SWN
                wd = min(SWN, VP - s0)
                ps, coff = spmm_sw(3, swi, layer_w[3], coff)
                x2 = slpool.tile([P, SWN], f32, tag="swsl")
                nc.vector.tensor_tensor(x2[:, :wd], ps[:, :wd],
                                        h2T[:, s0 : s0 + wd], sub)
                osl = slpool.tile([OUTF, SWN], f32, tag="osl")
                for t in range((wd + 511) // 512):
                    w512 = min(512, wd - t * 512)
                    pc = ps_d.tile([OUTF, 512], f32, tag="d")
                    rhss = [h2T[:, s0 + t * 512 : s0 + t * 512 + w512],
                            X1T[:, s0 + t * 512 : s0 + t * 512 + w512],
                            x2[:, t * 512 : t * 512 + w512]]
                    for k in range(3):
                        nc.tensor.matmul(out=pc[:, :w512], lhsT=Wcsb[:, k, :],
                                         rhs=rhss[k], start=(k == 0),
                                         stop=(k == 2))
                    nc.scalar.activation(osl[:, t * 512 : t * 512 + w512],
                                         pc[:, :w512], Ident, bias=b3sb[:])
                n128 = wd // P
                onm = nmpool.tile([P, SWN // P, OUTF], f32, tag="onm")
                for w8 in range(n128):
                    pt = ps_t.tile([P, P], f32, tag="t")
                    nc.tensor.transpose(out=pt[:, :OUTF],
                                        in_=osl[:, w8 * P : (w8 + 1) * P],
                                        identity=ident[:OUTF, :OUTF])
                    nc.vector.tensor_copy(onm[:, w8, :], pt[:, :OUTF])
                nc.sync.dma_start(
                    out_dram[s0 : s0 + wd, :].rearrange("(w p) f -> p w f", p=P),
                    onm[:, :n128, :])

    nc.compile()
    return nc


def _make_inputs(c_, percore, feat, W1, b1, W_self, W_neigh, b2, W_cheb, b3):
    NCORES, V, VP, SPAN = c_["NCORES"], c_["V"], c_["VP"], c_["SPAN"]
    OUTF, HID = c_["OUTF"], c_["HID"]
    feat = np.asarray(feat, np.float32)
    iota = np.tile(np.arange(SPAN, dtype=np.float32)[None, :], (P, 1))
    Wc3 = np.ascontiguousarray(
        np.asarray(W_cheb, np.float32).reshape(3, 128, OUTF).transpose(1, 0, 2))
    in_maps = []
    for c in range(NCORES):
        shard = np.zeros((VP, 128), np.float32)
        shard[:V] = feat[c * V : (c + 1) * V]
        pc = percore[c]
        in_maps.append(dict(
            feat_shard=shard, idx16=pc["idx16"], dstloc=pc["dstloc"],
            d0=pc["d0"], w0=pc["w0"], w1=pc["w1"], w2=pc["w2"], w3=pc["w3"],
            W1=np.asarray(W1, np.float32), W_self=np.asarray(W_self, np.float32),
            W_neigh=np.asarray(W_neigh, np.float32), W_cheb3=Wc3,
            b1=np.asarray(b1, np.float32).reshape(HID, 1),
            b2=np.asarray(b2, np.float32).reshape(HID, 1),
            b3=np.asarray(b3, np.float32).reshape(OUTF, 1),
            iota=iota,
        ))
    return in_maps


_CACHE = {}


def kernel(feat, src, dst, W1, b1, W_self, W_neigh, b2, W_cheb, b3):
    from concourse.bass_utils import run_bass_kernel_spmd

    c_, NCH, CT, percore = preprocess(src, dst, CFG)
    key = ("k", CT, tuple(tuple(r) for r in NCH))
    if key not in _CACHE:
        _CACHE[key] = build_kernel(c_, NCH, CT)
    nc = _CACHE[key]
    in_maps = _make_inputs(c_, percore, feat, W1, b1, W_self, W_neigh, b2,
                           W_cheb, b3)
    res = run_bass_kernel_spmd(nc, in_maps, core_ids=list(range(c_["NCORES"])))
    N, V, OUTF = c_["N"], c_["V"], c_["OUTF"]
    out = np.zeros((N, OUTF), np.float32)
    for c in range(c_["NCORES"]):
        out[c * V : (c + 1) * V] = res.results[c]["out"][:V]
    return out
